# revision 1
# baseline (speedup 1.0000x reference)
"""Trainium2 Bass kernel for nn_Connectivity3D (gnn_message_passing).

Same algebraic shortcuts as the original baseline (per-object mean GCN
collapse, bias/BN folding), with a rebuilt main loop:

  * bf16 weights/activations (fp32 PSUM), L1 bias folded in via a
    ones-row augmentation (K=14 block-diagonal over the 2 parts of an
    iteration)
  * 4-iteration software-pipeline skew: per loop body the PE stream is
    L1_{t+4}, L2_{t+3}, L3_t, so every instruction is ready on arrival
    and no engine FIFO ever parks on a semaphore
  * relu after L1 alternates between DVE (tensor_scalar) and ACT to
    balance the two PSUM-drain engines; maxpool is a per-half DVE
    reduce_max straight out of PSUM
  * PSUM: p1 [128,512]x2 + p2 [128,1024]x1 + p3 [128,1024]x2 = 8 banks

Sharding: data-parallel over objects; core k handles objects
[64k, 64k+64). Weights replicated.
"""

import numpy as np

NUM_OBJ = 512
K = 16
N_PARTS = NUM_OBJ * K        # 8192
P = 512                      # points per part
NCORES = 8
NLOC = N_PARTS // NCORES     # 1024 parts per core
OBJ_LOC = NLOC // K          # 64 objects per core
GROUPS = 128                 # groups per core
G_ITERS = 4                  # iters per group
G_PARTS = 8                  # parts per group

_prog_cache = {}


def _build_program(repeats=1, abl=()):
    import concourse.bass as bass
    import concourse.mybir as mybir
    import concourse.tile as tile
    from concourse import bacc
    from contextlib import ExitStack

    f32 = mybir.dt.float32
    bf16 = mybir.dt.bfloat16
    RELU = mybir.ActivationFunctionType.Relu
    IDENT = mybir.ActivationFunctionType.Identity
    TANH = mybir.ActivationFunctionType.Tanh
    AXX = mybir.AxisListType.X
    MAX = mybir.AluOpType.max

    nc = bacc.Bacc(trn_type="TRN2", target_bir_lowering=False)

    # ---- DRAM IO ----
    xt_d = nc.dram_tensor("xt", [GROUPS, 14, 2048], bf16, kind="ExternalInput")
    w1_d = nc.dram_tensor("w1a", [14, 128], bf16, kind="ExternalInput")
    w2_d = nc.dram_tensor("w2r", [128, 128], bf16, kind="ExternalInput")
    w3_d = nc.dram_tensor("w3s", [128, 256], bf16, kind="ExternalInput")
    b2_d = nc.dram_tensor("b2s", [128, 1], f32, kind="ExternalInput")
    wet_d = nc.dram_tensor("wet", [128, 512], f32, kind="ExternalInput")
    bet_d = nc.dram_tensor("bet", [128, 2], f32, kind="ExternalInput")
    wg1_d = nc.dram_tensor("wg1t", [128, 512], f32, kind="ExternalInput")
    bg1_d = nc.dram_tensor("bg1s", [128, 2], f32, kind="ExternalInput")
    wg2_d = nc.dram_tensor("wg2t", [128, 512], f32, kind="ExternalInput")
    bg2_d = nc.dram_tensor("bg2s", [128, 2], f32, kind="ExternalInput")
    wc1_d = nc.dram_tensor("wc1t", [128, 512], f32, kind="ExternalInput")
    bc1_d = nc.dram_tensor("bc1s", [128, 2], f32, kind="ExternalInput")
    wc2_d = nc.dram_tensor("wc2t", [128, 512], f32, kind="ExternalInput")
    bc2_d = nc.dram_tensor("bc2s", [128, 2], f32, kind="ExternalInput")
    wc3_d = nc.dram_tensor("wc3t", [128, 2], f32, kind="ExternalInput")
    bc3_d = nc.dram_tensor("bc3s", [1, 1], f32, kind="ExternalInput")
    msk_d = nc.dram_tensor("mask", [1, 256], f32, kind="ExternalInput")
    out_d = nc.dram_tensor("out", [OBJ_LOC, 256], f32, kind="ExternalOutput")

    with tile.TileContext(nc) as tc, ExitStack() as ctx:
        wp = ctx.enter_context(tc.tile_pool(name="wp", bufs=1))
        xp = ctx.enter_context(tc.tile_pool(name="xp", bufs=3))
        h1p = ctx.enter_context(tc.tile_pool(name="h1p", bufs=3))
        h2p = ctx.enter_context(tc.tile_pool(name="h2p", bufs=4))
        scrp = ctx.enter_context(tc.tile_pool(name="scrp", bufs=2))
        s2p = ctx.enter_context(tc.tile_pool(name="s2p", bufs=2))
        s3p = ctx.enter_context(tc.tile_pool(name="s3p", bufs=2))
        s4p = ctx.enter_context(tc.tile_pool(name="s4p", bufs=2))
        ftp = ctx.enter_context(tc.tile_pool(name="ftp", bufs=1))
        o2p = ctx.enter_context(tc.tile_pool(name="o2p", bufs=2))
        pp1 = ctx.enter_context(tc.tile_pool(name="pp1", bufs=2, space="PSUM"))
        pp2 = ctx.enter_context(tc.tile_pool(name="pp2", bufs=1, space="PSUM"))
        pp3 = ctx.enter_context(tc.tile_pool(name="pp3", bufs=2, space="PSUM"))

        def wload(dram, shape, dt):
            t = wp.tile(shape, dt, tag=dram.name, name=dram.name + "_s")
            nc.sync.dma_start(out=t[:], in_=dram[:])
            return t

        w1s = wload(w1_d, [14, 128], bf16)
        w2s = wload(w2_d, [128, 128], bf16)
        w3s = wload(w3_d, [128, 256], bf16)
        b2s = wload(b2_d, [128, 1], f32)
        wets = wload(wet_d, [128, 512], f32)
        bets = wload(bet_d, [128, 2], f32)
        wg1s = wload(wg1_d, [128, 512], f32)
        bg1s = wload(bg1_d, [128, 2], f32)
        wg2s = wload(wg2_d, [128, 512], f32)
        bg2s = wload(bg2_d, [128, 2], f32)
        wc1s = wload(wc1_d, [128, 512], f32)
        bc1s = wload(bc1_d, [128, 2], f32)
        wc2s = wload(wc2_d, [128, 512], f32)
        bc2s = wload(bc2_d, [128, 2], f32)
        wc3s = wload(wc3_d, [128, 2], f32)
        bc3s = wload(bc3_d, [1, 1], f32)
        msks = wload(msk_d, [1, 256], f32)

        # feat^T accumulators: feature half h x all parts of the core
        ft = [ftp.tile([128, NLOC], f32, tag=f"ft{h}", name=f"ft{h}") for h in range(2)]

        TOT = GROUPS * repeats * G_ITERS          # total iters
        xts = {}

        def ensure_dma(t):
            """DMA the group tile for global iter t if not yet issued."""
            gr = t // G_ITERS
            if gr in xts or gr >= GROUPS * repeats:
                return
            g = gr % GROUPS
            xt_t = xp.tile([14, 2048], bf16, tag="xt", name="xt_t")
            nc.sync.dma_start(out=xt_t[:], in_=xt_d[g, :, :])
            xts[gr] = xt_t

        def l1_act1(t):
            """L1 matmul + DVE relu for global iter t; returns h1 tile."""
            it = t % G_ITERS
            xt_t = xts[t // G_ITERS]
            p1t = pp1.tile([128, 512], f32, tag="p1", name="p1t")
            nc.tensor.matmul(
                p1t[:],
                lhsT=w1s[0:14, :],
                rhs=xt_t[0:14, 512 * it : 512 * it + 512],
            )
            h1 = h1p.tile([128, 512], bf16, tag="h1", name="h1t")
            if t % 2 == 0:
                nc.vector.tensor_scalar_max(h1[:], p1t[:], 0.0)
            else:
                nc.scalar.activation(h1[:], p1t[:], RELU)
            return h1

        def l2_act2(t, h1):
            """L2 matmuls + ACT relu-bias for iter t; returns h2 tile."""
            p2t = pp2.tile([128, 1024], f32, tag="p2", name="p2t")
            for s in range(2):
                nc.tensor.matmul(
                    p2t[:, 512 * s : 512 * s + 512],
                    lhsT=w2s[64 * s : 64 * s + 64, :],
                    rhs=h1[64 * s : 64 * s + 64, :],
                    tile_position=(64 * s, 0),
                )
            h2 = h2p.tile([128, 1024], bf16, tag="h2", name="h2t")
            nc.scalar.activation(h2[:], p2t[:], RELU, bias=b2s[:, 0:1])
            return h2

        def l3_red(t, h2):
            g = (t // G_ITERS) % GROUPS
            it = t % G_ITERS
            pc0 = G_PARTS * g + 2 * it
            for h in range(2):
                p3t = pp3.tile([128, 1024], f32, tag="p3", name="p3t")
                for s in range(2):
                    nc.tensor.matmul(
                        p3t[:, 512 * s : 512 * s + 512],
                        lhsT=w3s[:, 128 * h : 128 * h + 128],
                        rhs=h2[:, 512 * s : 512 * s + 512],
                    )
                nc.vector.reduce_max(
                    ft[h][:, pc0 : pc0 + 2],
                    p3t[:].rearrange("p (s q) -> p s q", q=512),
                    axis=AXX,
                )

        # 2-iteration software-pipeline skew: per body t the PE stream is
        # L1_{t+2}, L2_{t+1}, L3_t — every instruction's inputs were
        # produced >= 1 body earlier, so no engine ever parks on a wait.
        h1s, h2s = {}, {}
        for u in range(4):
            ensure_dma(u)
            h1s[u] = l1_act1(u)
        for u in range(3):
            h2s[u] = l2_act2(u, h1s.pop(u))
        for t in range(TOT):
            if t + 4 < TOT:
                ensure_dma(t + 4)
                h1s[t + 4] = l1_act1(t + 4)
            if t + 3 < TOT:
                h2s[t + 3] = l2_act2(t + 3, h1s.pop(t + 3))
            l3_red(t, h2s.pop(t))

        # ---- stage 2: per-object mean -> GCN x2 -> head -> output ----
        def dense256(win, bin_, src, func):
            outs = []
            for h in range(2):
                pe = pp1.tile([128, OBJ_LOC], f32, tag="p1", name="pe")
                for k in range(2):
                    nc.tensor.matmul(
                        pe[:],
                        lhsT=win[:, 256 * k + 128 * h : 256 * k + 128 * h + 128],
                        rhs=src[k][:],
                        start=(k == 0),
                        stop=(k == 1),
                    )
                o = o2p.tile([128, OBJ_LOC], f32, tag=f"s2_{id(win)}_{h}", name=f"s2o{h}")
                nc.scalar.activation(o[:], pe[:], func, bias=bin_[:, h : h + 1])
                outs.append(o)
            return outs

        sfeat = []
        for h in range(2):
            sf = o2p.tile([128, OBJ_LOC], f32, tag=f"sf{h}", name=f"sf{h}")
            nc.vector.reduce_sum(
                sf[:], ft[h][:].rearrange("p (o k) -> p o k", k=K), axis=AXX
            )
            sfeat.append(sf)

        memb = dense256(wets, bets, sfeat, IDENT)
        x1 = dense256(wg1s, bg1s, memb, RELU)
        z = dense256(wg2s, bg2s, x1, IDENT)
        c1 = dense256(wc1s, bc1s, z, RELU)
        c2 = dense256(wc2s, bc2s, c1, RELU)

        ps = pp2.tile([1, OBJ_LOC], f32, tag="p2", name="ps")
        for k in range(2):
            nc.tensor.matmul(
                ps[:], lhsT=wc3s[:, k : k + 1], rhs=c2[k][:],
                start=(k == 0), stop=(k == 1),
            )
        c_sb = o2p.tile([1, OBJ_LOC], f32, tag="c_sb", name="c_sb")
        nc.scalar.activation(c_sb[:], ps[:], TANH, bias=bc3s[:, 0:1])

        po = pp3.tile([OBJ_LOC, 256], f32, tag="p3", name="po")
        nc.tensor.matmul(po[:], lhsT=c_sb[:], rhs=msks[:])
        out_sb = o2p.tile([OBJ_LOC, 256], f32, tag="out_sb", name="out_sb")
        nc.scalar.copy(out_sb[:], po[:])
        nc.sync.dma_start(out=out_d[:], in_=out_sb[:])

    nc.compile()
    return nc


def _prep_inputs(inputs):
    """Fold BN/bias algebra on the host; build per-core input maps."""
    import ml_dtypes

    bf16 = ml_dtypes.bfloat16

    g = {k: np.asarray(v, np.float32) for k, v in inputs.items()
         if not k.startswith("edge")}

    W1f = g["W1"] * g["g1"][None, :]
    b1f = g["b1"] * g["g1"] + g["bt1"]
    W2f = g["W2"] * g["g2"][None, :]
    b2f = g["b2"] * g["g2"] + g["bt2"]
    W3f = g["W3"] * g["g3"][None, :]
    b3f = g["b3"] * g["g3"] + g["bt3"]

    wet = g["We"] / np.float32(K)
    bet = b3f @ g["We"] + g["be"]          # absorbs the L3 bias via the mean
    wc1f = g["Wc1"][:256] + g["Wc1"][256:]  # pair = [z, z] fold

    def tile256(W):
        return np.ascontiguousarray(
            W.reshape(2, 128, 2, 128).transpose(1, 0, 2, 3).reshape(128, 512)
        )

    def bias2(b):
        return np.ascontiguousarray(b.reshape(2, 128).T)

    w1a = np.zeros((14, 128), np.float32)
    w1a[0:6, 0:64] = W1f
    w1a[6, 0:64] = b1f
    w1a[7:13, 64:128] = W1f
    w1a[13, 64:128] = b1f
    w2r = np.vstack([W2f, W2f])

    mask = (1.0 - np.eye(K, dtype=np.float32)).reshape(1, 256)

    shared = {
        "w1a": w1a.astype(bf16), "w2r": w2r.astype(bf16),
        "w3s": W3f.astype(bf16),
        "b2s": np.ascontiguousarray(b2f[:, None]),
        "wet": tile256(wet), "bet": bias2(bet),
        "wg1t": tile256(g["Wg1"]), "bg1s": bias2(g["bg1"]),
        "wg2t": tile256(g["Wg2"]), "bg2s": bias2(g["bg2"]),
        "wc1t": tile256(wc1f), "bc1s": bias2(g["bc1"]),
        "wc2t": tile256(g["Wc2"]), "bc2s": bias2(g["bc2"]),
        "wc3t": np.ascontiguousarray(g["Wc3"].reshape(2, 128).T),
        "bc3s": g["bc3"].reshape(1, 1).astype(np.float32),
        "mask": mask,
    }

    pcls = np.asarray(inputs["pcls_arr"], np.float32)
    in_maps = []
    for k in range(NCORES):
        pc = pcls[k * NLOC : (k + 1) * NLOC]                   # [1024, 512, 6]
        arr = (
            pc.reshape(GROUPS, G_ITERS, 2, P, 6)
            .transpose(0, 2, 4, 1, 3)
            .reshape(GROUPS, 2, 6, 2048)
            .astype(bf16)
        )
        xt = np.ones((GROUPS, 14, 2048), bf16)
        xt[:, 0:6] = arr[:, 0]
        xt[:, 7:13] = arr[:, 1]
        m = dict(shared)
        m["xt"] = xt
        in_maps.append(m)
    return in_maps


def _get_prog():
    if "nc" not in _prog_cache:
        _prog_cache["nc"] = _build_program()
    return _prog_cache["nc"]


def _run(inputs, trace=False, **kw):
    from concourse.bass_utils import run_bass_kernel_spmd

    nc = _get_prog()
    in_maps = _prep_inputs(inputs)
    res = run_bass_kernel_spmd(
        nc, in_maps, core_ids=list(range(NCORES)), trace=trace, **kw
    )
    outs = [r["out"].reshape(OBJ_LOC, K, K) for r in res.results]
    full = np.concatenate(outs, axis=0).astype(np.float32)
    return full, res


def kernel(**inputs) -> np.ndarray:
    out, _ = _run(inputs, trace=False)
    return out


def bench(inputs, **kw):
    return _run(inputs, trace=True, **kw)



# revision 7
# speedup vs baseline: 1.1095x; 1.1095x over previous
"""Trainium2 Bass kernel for nn_Connectivity3D (gnn_message_passing).

Same algebraic shortcuts as the original baseline (per-object mean GCN
collapse, bias/BN folding), with a rebuilt main loop:

  * bf16 weights/activations (fp32 PSUM), L1 bias folded in via a
    ones-row augmentation (K=14 block-diagonal over the 2 parts of an
    iteration)
  * 4-iteration software-pipeline skew: per loop body the PE stream is
    L1_{t+4}, L2_{t+3}, L3_t, so every instruction is ready on arrival
    and no engine FIFO ever parks on a semaphore
  * relu after L1 alternates between DVE (tensor_scalar) and ACT to
    balance the two PSUM-drain engines; maxpool is a per-half DVE
    reduce_max straight out of PSUM
  * PSUM: p1 [128,512]x2 + p2 [128,1024]x1 + p3 [128,1024]x2 = 8 banks

Sharding: data-parallel over objects; core k handles objects
[64k, 64k+64). Weights replicated.
"""

import numpy as np

NUM_OBJ = 512
K = 16
N_PARTS = NUM_OBJ * K        # 8192
P = 512                      # points per part
NCORES = 8
NLOC = N_PARTS // NCORES     # 1024 parts per core
OBJ_LOC = NLOC // K          # 64 objects per core
GROUPS = 128                 # groups per core
G_ITERS = 4                  # iters per group
G_PARTS = 8                  # parts per group

_prog_cache = {}


def _build_program(repeats=1, abl=()):
    import concourse.bass as bass
    import concourse.mybir as mybir
    import concourse.tile as tile
    from concourse import bacc
    from contextlib import ExitStack

    f32 = mybir.dt.float32
    bf16 = mybir.dt.bfloat16
    RELU = mybir.ActivationFunctionType.Relu
    IDENT = mybir.ActivationFunctionType.Identity
    TANH = mybir.ActivationFunctionType.Tanh
    AXX = mybir.AxisListType.X
    MAX = mybir.AluOpType.max

    nc = bacc.Bacc(trn_type="TRN2", target_bir_lowering=False)

    # ---- DRAM IO ----
    xt_d = nc.dram_tensor("xt", [GROUPS, 14, 2048], bf16, kind="ExternalInput")
    w1_d = nc.dram_tensor("w1a", [14, 128], bf16, kind="ExternalInput")
    w2_d = nc.dram_tensor("w2r", [128, 128], bf16, kind="ExternalInput")
    w3_d = nc.dram_tensor("w3s", [128, 256], bf16, kind="ExternalInput")
    b2_d = nc.dram_tensor("b2s", [128, 1], f32, kind="ExternalInput")
    wet_d = nc.dram_tensor("wet", [128, 512], f32, kind="ExternalInput")
    bet_d = nc.dram_tensor("bet", [128, 2], f32, kind="ExternalInput")
    wg1_d = nc.dram_tensor("wg1t", [128, 512], f32, kind="ExternalInput")
    bg1_d = nc.dram_tensor("bg1s", [128, 2], f32, kind="ExternalInput")
    wg2_d = nc.dram_tensor("wg2t", [128, 512], f32, kind="ExternalInput")
    bg2_d = nc.dram_tensor("bg2s", [128, 2], f32, kind="ExternalInput")
    wc1_d = nc.dram_tensor("wc1t", [128, 512], f32, kind="ExternalInput")
    bc1_d = nc.dram_tensor("bc1s", [128, 2], f32, kind="ExternalInput")
    wc2_d = nc.dram_tensor("wc2t", [128, 512], f32, kind="ExternalInput")
    bc2_d = nc.dram_tensor("bc2s", [128, 2], f32, kind="ExternalInput")
    wc3_d = nc.dram_tensor("wc3t", [128, 2], f32, kind="ExternalInput")
    bc3_d = nc.dram_tensor("bc3s", [1, 1], f32, kind="ExternalInput")
    msk_d = nc.dram_tensor("mask", [1, 256], f32, kind="ExternalInput")
    out_d = nc.dram_tensor("out", [OBJ_LOC, 256], f32, kind="ExternalOutput")

    with tile.TileContext(nc) as tc, ExitStack() as ctx:
        wp = ctx.enter_context(tc.tile_pool(name="wp", bufs=1))
        xp = ctx.enter_context(tc.tile_pool(name="xp", bufs=3))
        h1p = ctx.enter_context(tc.tile_pool(name="h1p", bufs=3))
        h2p = ctx.enter_context(tc.tile_pool(name="h2p", bufs=4))
        scrp = ctx.enter_context(tc.tile_pool(name="scrp", bufs=2))
        s2p = ctx.enter_context(tc.tile_pool(name="s2p", bufs=2))
        s3p = ctx.enter_context(tc.tile_pool(name="s3p", bufs=2))
        s4p = ctx.enter_context(tc.tile_pool(name="s4p", bufs=2))
        ftp = ctx.enter_context(tc.tile_pool(name="ftp", bufs=1))
        o2p = ctx.enter_context(tc.tile_pool(name="o2p", bufs=2))
        pp1 = ctx.enter_context(tc.tile_pool(name="pp1", bufs=2, space="PSUM"))
        pp2 = ctx.enter_context(tc.tile_pool(name="pp2", bufs=1, space="PSUM"))
        pp3 = ctx.enter_context(tc.tile_pool(name="pp3", bufs=2, space="PSUM"))

        def wload(dram, shape, dt):
            t = wp.tile(shape, dt, tag=dram.name, name=dram.name + "_s")
            nc.sync.dma_start(out=t[:], in_=dram[:])
            return t

        w1s = wload(w1_d, [14, 128], bf16)
        w2s = wload(w2_d, [128, 128], bf16)
        w3s = wload(w3_d, [128, 256], bf16)
        b2s = wload(b2_d, [128, 1], f32)
        wets = wload(wet_d, [128, 512], f32)
        bets = wload(bet_d, [128, 2], f32)
        wg1s = wload(wg1_d, [128, 512], f32)
        bg1s = wload(bg1_d, [128, 2], f32)
        wg2s = wload(wg2_d, [128, 512], f32)
        bg2s = wload(bg2_d, [128, 2], f32)
        wc1s = wload(wc1_d, [128, 512], f32)
        bc1s = wload(bc1_d, [128, 2], f32)
        wc2s = wload(wc2_d, [128, 512], f32)
        bc2s = wload(bc2_d, [128, 2], f32)
        wc3s = wload(wc3_d, [128, 2], f32)
        bc3s = wload(bc3_d, [1, 1], f32)
        msks = wload(msk_d, [1, 256], f32)

        # feat^T accumulators: feature half h x all parts of the core
        ft = [ftp.tile([128, NLOC], f32, tag=f"ft{h}", name=f"ft{h}") for h in range(2)]

        TOT = GROUPS * repeats * G_ITERS          # total iters
        xts = {}

        def ensure_dma(t):
            """DMA the group tile for global iter t if not yet issued."""
            gr = t // G_ITERS
            if gr in xts or gr >= GROUPS * repeats:
                return
            g = gr % GROUPS
            xt_t = xp.tile([14, 2048], bf16, tag="xt", name="xt_t")
            nc.sync.dma_start(out=xt_t[:], in_=xt_d[g, :, :])
            xts[gr] = xt_t

        def l1_act1(t):
            """L1 matmul + ACT relu for global iter t; returns h1 tile.

            Both relus live on ACT so the DVE stream is nothing but the two
            maxpool reduces per iteration — the DVE is the drain bottleneck
            (PSUM reads are 1 elem/cycle/lane on every engine, no fast
            modes), so any non-reduce work on it directly lengthens the
            critical path.
            """
            it = t % G_ITERS
            xt_t = xts[t // G_ITERS]
            p1t = pp1.tile([128, 512], f32, tag="p1", name="p1t")
            nc.tensor.matmul(
                p1t[:],
                lhsT=w1s[0:14, :],
                rhs=xt_t[0:14, 512 * it : 512 * it + 512],
            )
            h1 = h1p.tile([128, 512], bf16, tag="h1", name="h1t")
            nc.scalar.activation(h1[:], p1t[:], RELU)
            return h1

        def l2_act2(t, h1):
            """L2 matmuls + ACT relu-bias for iter t; returns h2 tile."""
            p2t = pp2.tile([128, 1024], f32, tag="p2", name="p2t")
            for s in range(2):
                nc.tensor.matmul(
                    p2t[:, 512 * s : 512 * s + 512],
                    lhsT=w2s[64 * s : 64 * s + 64, :],
                    rhs=h1[64 * s : 64 * s + 64, :],
                    tile_position=(64 * s, 0),
                )
            h2 = h2p.tile([128, 1024], bf16, tag="h2", name="h2t")
            nc.scalar.activation(h2[:], p2t[:], RELU, bias=b2s[:, 0:1])
            return h2

        def l3_red(t, h2):
            g = (t // G_ITERS) % GROUPS
            it = t % G_ITERS
            pc0 = G_PARTS * g + 2 * it
            for h in range(2):
                p3t = pp3.tile([128, 1024], f32, tag="p3", name="p3t")
                for s in range(2):
                    nc.tensor.matmul(
                        p3t[:, 512 * s : 512 * s + 512],
                        lhsT=w3s[:, 128 * h : 128 * h + 128],
                        rhs=h2[:, 512 * s : 512 * s + 512],
                    )
                nc.vector.reduce_max(
                    ft[h][:, pc0 : pc0 + 2],
                    p3t[:].rearrange("p (s q) -> p s q", q=512),
                    axis=AXX,
                )

        # 2-iteration software-pipeline skew: per body t the PE stream is
        # L1_{t+2}, L2_{t+1}, L3_t — every instruction's inputs were
        # produced >= 1 body earlier, so no engine ever parks on a wait.
        h1s, h2s = {}, {}
        for u in range(4):
            ensure_dma(u)
            h1s[u] = l1_act1(u)
        for u in range(3):
            h2s[u] = l2_act2(u, h1s.pop(u))
        for t in range(TOT):
            if t + 4 < TOT:
                ensure_dma(t + 4)
                h1s[t + 4] = l1_act1(t + 4)
            if t + 3 < TOT:
                h2s[t + 3] = l2_act2(t + 3, h1s.pop(t + 3))
            l3_red(t, h2s.pop(t))

        # ---- stage 2: per-object mean -> GCN x2 -> head -> output ----
        def dense256(win, bin_, src, func):
            outs = []
            for h in range(2):
                pe = pp1.tile([128, OBJ_LOC], f32, tag="p1", name="pe")
                for k in range(2):
                    nc.tensor.matmul(
                        pe[:],
                        lhsT=win[:, 256 * k + 128 * h : 256 * k + 128 * h + 128],
                        rhs=src[k][:],
                        start=(k == 0),
                        stop=(k == 1),
                    )
                o = o2p.tile([128, OBJ_LOC], f32, tag=f"s2_{id(win)}_{h}", name=f"s2o{h}")
                nc.scalar.activation(o[:], pe[:], func, bias=bin_[:, h : h + 1])
                outs.append(o)
            return outs

        sfeat = []
        for h in range(2):
            sf = o2p.tile([128, OBJ_LOC], f32, tag=f"sf{h}", name=f"sf{h}")
            nc.vector.reduce_sum(
                sf[:], ft[h][:].rearrange("p (o k) -> p o k", k=K), axis=AXX
            )
            sfeat.append(sf)

        memb = dense256(wets, bets, sfeat, IDENT)
        x1 = dense256(wg1s, bg1s, memb, RELU)
        z = dense256(wg2s, bg2s, x1, IDENT)
        c1 = dense256(wc1s, bc1s, z, RELU)
        c2 = dense256(wc2s, bc2s, c1, RELU)

        ps = pp2.tile([1, OBJ_LOC], f32, tag="p2", name="ps")
        for k in range(2):
            nc.tensor.matmul(
                ps[:], lhsT=wc3s[:, k : k + 1], rhs=c2[k][:],
                start=(k == 0), stop=(k == 1),
            )
        c_sb = o2p.tile([1, OBJ_LOC], f32, tag="c_sb", name="c_sb")
        nc.scalar.activation(c_sb[:], ps[:], TANH, bias=bc3s[:, 0:1])

        po = pp3.tile([OBJ_LOC, 256], f32, tag="p3", name="po")
        nc.tensor.matmul(po[:], lhsT=c_sb[:], rhs=msks[:])
        out_sb = o2p.tile([OBJ_LOC, 256], f32, tag="out_sb", name="out_sb")
        nc.scalar.copy(out_sb[:], po[:])
        nc.sync.dma_start(out=out_d[:], in_=out_sb[:])

    nc.compile()
    return nc


def _prep_inputs(inputs):
    """Fold BN/bias algebra on the host; build per-core input maps."""
    import ml_dtypes

    bf16 = ml_dtypes.bfloat16

    g = {k: np.asarray(v, np.float32) for k, v in inputs.items()
         if not k.startswith("edge")}

    W1f = g["W1"] * g["g1"][None, :]
    b1f = g["b1"] * g["g1"] + g["bt1"]
    W2f = g["W2"] * g["g2"][None, :]
    b2f = g["b2"] * g["g2"] + g["bt2"]
    W3f = g["W3"] * g["g3"][None, :]
    b3f = g["b3"] * g["g3"] + g["bt3"]

    wet = g["We"] / np.float32(K)
    bet = b3f @ g["We"] + g["be"]          # absorbs the L3 bias via the mean
    wc1f = g["Wc1"][:256] + g["Wc1"][256:]  # pair = [z, z] fold

    def tile256(W):
        return np.ascontiguousarray(
            W.reshape(2, 128, 2, 128).transpose(1, 0, 2, 3).reshape(128, 512)
        )

    def bias2(b):
        return np.ascontiguousarray(b.reshape(2, 128).T)

    w1a = np.zeros((14, 128), np.float32)
    w1a[0:6, 0:64] = W1f
    w1a[6, 0:64] = b1f
    w1a[7:13, 64:128] = W1f
    w1a[13, 64:128] = b1f
    w2r = np.vstack([W2f, W2f])

    mask = (1.0 - np.eye(K, dtype=np.float32)).reshape(1, 256)

    shared = {
        "w1a": w1a.astype(bf16), "w2r": w2r.astype(bf16),
        "w3s": W3f.astype(bf16),
        "b2s": np.ascontiguousarray(b2f[:, None]),
        "wet": tile256(wet), "bet": bias2(bet),
        "wg1t": tile256(g["Wg1"]), "bg1s": bias2(g["bg1"]),
        "wg2t": tile256(g["Wg2"]), "bg2s": bias2(g["bg2"]),
        "wc1t": tile256(wc1f), "bc1s": bias2(g["bc1"]),
        "wc2t": tile256(g["Wc2"]), "bc2s": bias2(g["bc2"]),
        "wc3t": np.ascontiguousarray(g["Wc3"].reshape(2, 128).T),
        "bc3s": g["bc3"].reshape(1, 1).astype(np.float32),
        "mask": mask,
    }

    pcls = np.asarray(inputs["pcls_arr"], np.float32)
    in_maps = []
    for k in range(NCORES):
        pc = pcls[k * NLOC : (k + 1) * NLOC]                   # [1024, 512, 6]
        arr = (
            pc.reshape(GROUPS, G_ITERS, 2, P, 6)
            .transpose(0, 2, 4, 1, 3)
            .reshape(GROUPS, 2, 6, 2048)
            .astype(bf16)
        )
        xt = np.ones((GROUPS, 14, 2048), bf16)
        xt[:, 0:6] = arr[:, 0]
        xt[:, 7:13] = arr[:, 1]
        m = dict(shared)
        m["xt"] = xt
        in_maps.append(m)
    return in_maps


def _get_prog():
    if "nc" not in _prog_cache:
        _prog_cache["nc"] = _build_program()
    return _prog_cache["nc"]


def _run(inputs, trace=False, **kw):
    from concourse.bass_utils import run_bass_kernel_spmd

    nc = _get_prog()
    in_maps = _prep_inputs(inputs)
    res = run_bass_kernel_spmd(
        nc, in_maps, core_ids=list(range(NCORES)), trace=trace, **kw
    )
    outs = [r["out"].reshape(OBJ_LOC, K, K) for r in res.results]
    full = np.concatenate(outs, axis=0).astype(np.float32)
    return full, res


def kernel(**inputs) -> np.ndarray:
    out, _ = _run(inputs, trace=False)
    return out


def bench(inputs, **kw):
    return _run(inputs, trace=True, **kw)

